# revision 2
# baseline (speedup 1.0000x reference)
"""DescriptorRetentionLoss on 8 Trainium2 cores — window-pruned version.

Matches are within 2px in a 640x480 image, so they are spatially local in x.
Host sorts both point sets by x, gives core c the c-th x-strip of 512 valid
rows plus only the memory columns whose x falls inside the strip +-3px
(~1100 of 8192; padded to CAP=1536).  All terms of the loss that are not
pair-local are low-rank and finish on the host:

  loss = (S@T + A) / (max(n_pairs,1) * max(n_rows,1))
    S[m]  column match counts        -> device (ones @ mask), host scatter-add
    T[m]  = (W . yhat_m),  W = sum_i rh_i*rx_i*x_i  -> device emits per-core
            w = g @ x (bf16); host does the [M,D]@[D] matvec on yhat.
    A     = sum(rc) - 2*sum(mc); mc_i = sum_m mask*cos -> device amr
  cos[i,m] = (x_i . yhat_m*16) * (rx_i/16): memory descriptors are normalized
  on the host and shipped fp8e4m3 (scaled by 16); x descriptors fp8e4m3; the
  descriptor matmul runs in DoubleRow fp8 mode (0.5 cyc/row).

The exact-fp32 distance mask uses the baseline's split-11 trick unchanged:
d2 is computed to ~2^-11 ulp accuracy via an 11-row f32r matmul and compared
against thr = 4 - |x|^2 (f64 on host).
"""

import sys

sys.path.insert(0, "/opt/trn_rl_repo")

import numpy as np
from contextlib import ExitStack

N, M, D = 4096, 8192, 512
NCORES = 8
NL = N // NCORES          # 512 rows per core
NT = NL // 128            # 4 row tiles
KC = D // 128             # 4 contraction chunks
CAP = 1536                # candidate memory columns per core (padded)
MSUB = 512                # column block
MJ = CAP // MSUB          # 3 column blocks
MARGIN = 3.0              # px window margin; safe vs fp32 d2 rounding
PADY = 1.0e6              # x coord for padding columns (never matches)

_cached = {}


def _split11(v):
    """Exact 2-piece split of fp32 into <=11-mantissa-bit halves."""
    v = np.asarray(v, np.float32)
    m, e = np.frexp(v)
    hi = np.ldexp(np.trunc(np.ldexp(m, 11)), e - 11).astype(np.float32)
    return hi, (v - hi).astype(np.float32)


def _split11_multi(v64, n):
    pieces = []
    rem = np.asarray(v64, np.float64)
    for _ in range(n):
        r32 = rem.astype(np.float32)
        m, e = np.frexp(r32)
        hi = np.ldexp(np.trunc(np.ldexp(m, 11)), e - 11).astype(np.float32)
        pieces.append(hi)
        rem = rem - hi.astype(np.float64)
    return pieces


def _mk_xpts(xp):
    x0h, x0l = _split11(xp[:, 0])
    x1h, x1l = _split11(xp[:, 1])
    one = np.ones(xp.shape[0], np.float32)
    # row k pairs with ypts row k: [y0h,y0l,y0h,y0l,y1h,y1l,y1h,y1l,yy1,yy2,yy3]
    return np.ascontiguousarray(np.stack(
        [-2 * x0h, -2 * x0h, -2 * x0l, -2 * x0l,
         -2 * x1h, -2 * x1h, -2 * x1l, -2 * x1l, one, one, one]))


def _mk_ypts(yp):
    y0h, y0l = _split11(yp[:, 0])
    y1h, y1l = _split11(yp[:, 1])
    yy64 = (yp[:, 0].astype(np.float64) ** 2 + yp[:, 1].astype(np.float64) ** 2)
    yy1, yy2, yy3 = _split11_multi(yy64, 3)
    return np.ascontiguousarray(
        np.stack([y0h, y0l, y0h, y0l, y1h, y1l, y1h, y1l, yy1, yy2, yy3]))


def _f8():
    import ml_dtypes
    return ml_dtypes.float8_e4m3fn


def _bf16():
    import ml_dtypes
    return ml_dtypes.bfloat16


def _build_nc():
    from concourse import bacc, bass, mybir, tile

    f32 = mybir.dt.float32
    f32r = mybir.dt.float32r
    bf16 = mybir.dt.bfloat16
    f8e4 = mybir.dt.float8e4
    nc = bacc.Bacc("TRN2", target_bir_lowering=False, debug=False)

    xdT8 = nc.dram_tensor("xdT8", [D, NL], f8e4, kind="ExternalInput")
    xnat = nc.dram_tensor("xnat", [NL, D], bf16, kind="ExternalInput")
    xpts = nc.dram_tensor("xpts", [11, NL], f32, kind="ExternalInput")
    thr = nc.dram_tensor("thr", [NL], f32, kind="ExternalInput")
    rx16 = nc.dram_tensor("rx16", [NL], f32, kind="ExternalInput")
    ypts = nc.dram_tensor("ypts", [11, CAP], f32, kind="ExternalInput")
    ydT8 = nc.dram_tensor("ydT8", [D, CAP], f8e4, kind="ExternalInput")

    S_out = nc.dram_tensor("S_out", [CAP], f32, kind="ExternalOutput")
    rc_out = nc.dram_tensor("rc_out", [NL], f32, kind="ExternalOutput")
    mc_out = nc.dram_tensor("mc_out", [NL], f32, kind="ExternalOutput")
    w_out = nc.dram_tensor("w_out", [D], f32, kind="ExternalOutput")

    AF = mybir.ActivationFunctionType
    OP = mybir.AluOpType
    X = mybir.AxisListType.X
    DR = mybir.MatmulPerfMode.DoubleRow

    with ExitStack() as ctx:
        tc = ctx.enter_context(tile.TileContext(nc))
        singles = ctx.enter_context(tc.tile_pool(name="singles", bufs=1))
        yd_pool = ctx.enter_context(tc.tile_pool(name="ydp", bufs=2))
        yp_pool = ctx.enter_context(tc.tile_pool(name="ypp", bufs=2))
        mf_pool = ctx.enter_context(tc.tile_pool(name="mfp", bufs=8))
        small = ctx.enter_context(tc.tile_pool(name="small", bufs=8))
        evac_pool = ctx.enter_context(tc.tile_pool(name="evac", bufs=4))
        ps_p = ctx.enter_context(tc.tile_pool(name="ps_p", bufs=2, space="PSUM"))
        ps_d = ctx.enter_context(tc.tile_pool(name="ps_d", bufs=2, space="PSUM"))
        ps_s = ctx.enter_context(tc.tile_pool(name="ps_s", bufs=2, space="PSUM"))

        ones = singles.tile([128, 1], bf16)
        nc.vector.memset(ones, 1.0)

        # ---- static loads ----
        sxdT = singles.tile([128, KC, NL], f8e4)
        nc.sync.dma_start(out=sxdT,
                          in_=xdT8[:, :].rearrange("(c p) n -> p c n", p=128))
        sxn = singles.tile([128, NT, D], bf16)
        nc.sync.dma_start(out=sxn,
                          in_=xnat[:, :].rearrange("(t p) d -> p t d", p=128))
        sxpts = singles.tile([11, NL], f32r)
        nc.gpsimd.dma_start(out=sxpts, in_=xpts[:, :])
        sthr = singles.tile([128, NT], f32)
        nc.gpsimd.dma_start(out=sthr, in_=thr.rearrange("(t p) -> p t", p=128))
        srx = singles.tile([128, NT], f32)
        nc.gpsimd.dma_start(out=srx, in_=rx16.rearrange("(t p) -> p t", p=128))

        rcst = singles.tile([128, NT, MJ], f32)
        mcst = singles.tile([128, NT, MJ], f32)

        # ---- main loop over column blocks ----
        for j in range(MJ):
            syd = yd_pool.tile([128, KC, MSUB], f8e4, name=f"syd{j}", tag="syd")
            nc.sync.dma_start(
                out=syd,
                in_=ydT8[:, j * MSUB:(j + 1) * MSUB].rearrange(
                    "(c p) m -> p c m", p=128))
            syp = yp_pool.tile([11, MSUB], f32r, name=f"syp{j}", tag="syp")
            nc.gpsimd.dma_start(out=syp, in_=ypts[:, j * MSUB:(j + 1) * MSUB])

            mfl = []
            for t in range(NT):
                pp = ps_p.tile([128, MSUB], f32, name=f"pp{j}_{t}", tag="pp")
                nc.tensor.matmul(pp, sxpts[:, t * 128:(t + 1) * 128], syp,
                                 start=True, stop=True)
                mf = mf_pool.tile([128, MSUB], bf16, name=f"mf{j}_{t}", tag="mf")
                nc.vector.tensor_scalar(
                    out=mf, in0=pp, scalar1=sthr[:, t:t + 1], scalar2=None,
                    op0=OP.is_lt, op1=OP.add,
                    accum_out=rcst[:, t, j:j + 1])
                mfl.append(mf)

            for t in range(NT):
                pd = ps_d.tile([128, MSUB], f32, name=f"pd{j}_{t}", tag="pd")
                nc.tensor.matmul(pd, sxdT[:, 0:2, t * 128:(t + 1) * 128],
                                 syd[:, 0:2, :], start=True, stop=False,
                                 perf_mode=DR)
                nc.tensor.matmul(pd, sxdT[:, 2:4, t * 128:(t + 1) * 128],
                                 syd[:, 2:4, :], start=False, stop=True,
                                 perf_mode=DR)
                dummy = small.tile([128, 1], f32, name=f"dm{j}_{t}", tag="dm")
                nc.vector.affine_mul_reduce(
                    out=dummy.broadcast_to(pd.shape),
                    accum_out=mcst[:, t, j:j + 1],
                    in0=pd, in1=mfl[t], scale=srx[:, t:t + 1], bias=0.0)

            pS = ps_s.tile([1, MSUB], f32, name=f"pS{j}", tag="pS")
            for t in range(NT):
                nc.tensor.matmul(pS, ones, mfl[t], start=(t == 0),
                                 stop=(t == NT - 1))
            sS = evac_pool.tile([1, MSUB], f32, name=f"sS{j}", tag="sS")
            nc.scalar.activation(sS, pS, AF.Copy)
            nc.sync.dma_start(out=S_out[j * MSUB:(j + 1) * MSUB], in_=sS)

        # ---- row stats + w ----
        rc_row = singles.tile([128, NT], f32)
        for t in range(NT):
            nc.vector.tensor_reduce(out=rc_row[:, t:t + 1], in_=rcst[:, t, :],
                                    axis=X, op=OP.add)
        nc.sync.dma_start(out=rc_out.rearrange("(t p) -> p t", p=128), in_=rc_row)

        rh = small.tile([128, NT], f32, name="rh", tag="rh")
        nc.vector.tensor_scalar(out=rh, in0=rc_row, scalar1=0.0, scalar2=None,
                                op0=OP.is_gt)
        g = small.tile([128, NT], f32, name="g", tag="g")
        nc.vector.tensor_tensor(g, rh, srx, op=OP.mult)
        gb = small.tile([128, NT], bf16, name="gb", tag="gb")
        nc.scalar.copy(gb, g)

        pW = ps_s.tile([1, D], f32, name="pW", tag="pS")
        for t in range(NT):
            nc.tensor.matmul(pW, gb[:, t:t + 1], sxn[:, t, :], start=(t == 0),
                             stop=(t == NT - 1))
        sW = evac_pool.tile([1, D], f32, name="sW", tag="sS")
        nc.scalar.activation(sW, pW, AF.Copy)
        nc.sync.dma_start(out=w_out[:], in_=sW)

        mc_row = singles.tile([128, NT], f32)
        for t in range(NT):
            nc.vector.tensor_reduce(out=mc_row[:, t:t + 1], in_=mcst[:, t, :],
                                    axis=X, op=OP.add)
        nc.sync.dma_start(out=mc_out.rearrange("(t p) -> p t", p=128), in_=mc_row)

    nc.finalize()
    return nc


def _get_nc():
    if "nc" not in _cached:
        _cached["nc"] = _build_nc()
    return _cached["nc"]


def _make_in_maps(valid_pts_scr, mem_pts_scr, valid_desc, mem_desc):
    """Returns (in_maps, meta); meta carries window slices + host-side y data."""
    f8 = _f8()
    bf = _bf16()
    vp = np.asarray(valid_pts_scr, np.float32)
    mp = np.asarray(mem_pts_scr, np.float32)
    vd = np.asarray(valid_desc, np.float32)
    md = np.asarray(mem_desc, np.float32)

    xs = np.argsort(vp[:, 0], kind="stable")
    ms = np.argsort(mp[:, 0], kind="stable")
    vp_s, vd_s = vp[xs], vd[xs]
    mp_s, md_s = mp[ms], md[ms]

    yy = np.sum(md_s.astype(np.float64) ** 2, -1)
    ry = 1.0 / np.sqrt(yy)
    yhat = (md_s * ry[:, None].astype(np.float64)).astype(np.float32)  # [M, D]
    ydT8_full = np.ascontiguousarray((yhat * 16.0).T.astype(f8))       # [D, M]
    ypts_full = _mk_ypts(mp_s)                                         # [11, M]

    xx_pt = (vp_s[:, 0].astype(np.float64) ** 2
             + vp_s[:, 1].astype(np.float64) ** 2)
    xx_d = np.sum(vd_s.astype(np.float64) ** 2, -1)
    rx = 1.0 / np.sqrt(xx_d)

    pad_pts = np.full((1, 2), PADY, np.float32)
    ypts_pad = _mk_ypts(pad_pts)[:, 0]                                 # [11]

    in_maps, slices = [], []
    for c in range(NCORES):
        rows = slice(c * NL, (c + 1) * NL)
        xp = vp_s[rows]
        lo = xp[:, 0].min() - MARGIN
        hi = xp[:, 0].max() + MARGIN
        s = int(np.searchsorted(mp_s[:, 0], lo, side="left"))
        e = int(np.searchsorted(mp_s[:, 0], hi, side="right"))
        n_c = e - s
        assert n_c <= CAP, f"core {c}: window {n_c} exceeds CAP {CAP}"
        slices.append((s, e))

        ypts_c = np.empty((11, CAP), np.float32)
        ypts_c[:, :n_c] = ypts_full[:, s:e]
        ypts_c[:, n_c:] = ypts_pad[:, None]
        ydT8_c = np.zeros((D, CAP), f8)
        ydT8_c[:, :n_c] = ydT8_full[:, s:e]

        in_maps.append({
            "xdT8": np.ascontiguousarray(vd_s[rows].T.astype(f8)),
            "xnat": np.ascontiguousarray(vd_s[rows].astype(bf)),
            "xpts": _mk_xpts(xp),
            "thr": (4.0 - xx_pt[rows]).astype(np.float32),
            "rx16": (rx[rows] / 16.0).astype(np.float32),
            "ypts": np.ascontiguousarray(ypts_c),
            "ydT8": np.ascontiguousarray(ydT8_c),
        })
    meta = {"slices": slices, "yhat": yhat}
    return in_maps, meta


def _finish(results, meta):
    slices = meta["slices"]
    yhat = meta["yhat"]
    Stot = np.zeros(M, np.float64)
    A = 0.0
    nrows = 0.0
    W = np.zeros(D, np.float64)
    for c in range(NCORES):
        r = results[c]
        s, e = slices[c]
        Stot[s:e] += r["S_out"].astype(np.float64)[:e - s]
        rc = r["rc_out"].astype(np.float64)
        mc = r["mc_out"].astype(np.float64)
        A += rc.sum() - 2.0 * mc.sum()
        nrows += float((rc > 0).sum())
        W += r["w_out"].astype(np.float64) * 16.0
    npairs = Stot.sum()
    if nrows > 0:
        T = yhat.astype(np.float64) @ W
        loss = (Stot @ T + A) / (max(npairs, 1.0) * max(nrows, 1.0))
    else:
        loss = 0.0
    return np.float32(loss)


def kernel(valid_pts_scr, mem_pts_scr, valid_desc, mem_desc):
    from concourse.bass_utils import run_bass_kernel_spmd

    in_maps, meta = _make_in_maps(valid_pts_scr, mem_pts_scr,
                                  valid_desc, mem_desc)
    nc = _get_nc()
    res = run_bass_kernel_spmd(nc, in_maps, core_ids=list(range(NCORES)))
    _cached["last_results"] = res
    return _finish(res.results, meta)


# revision 4
# speedup vs baseline: 1.8932x; 1.8932x over previous
"""DescriptorRetentionLoss on 8 Trainium2 cores — per-tile window version.

Matches are within 2px in a 640x480 image, so they are spatially local in x.
The host sorts both point sets by x; core c gets the c-th x-strip of 512
valid rows.  Within a core, each 128-row tile t spans only ~20px, so its
candidate memory columns (strip +-3px) fit in a WT=512 window of the sorted
memory arrays (measured max 395).  Windows are real column slices (clamped,
padded with real neighbors), so every computed quantity is exact.

Per (core, tile): an 11-row split-f32 matmul reproduces the reference's fp32
d2 to ~2^-11 ulp; DVE thresholds it against thr = 4 - |x|^2 (mask + row
counts), the fp8e4m3 DoubleRow descriptor matmul gives x . yhat*16, and one
affine_mul_reduce accumulates mc_i = sum_m mask * cos.  A [4, WT] PSUM tile
accumulates per-tile column counts via one-hot column weights.

Everything quadratic stays on device; the host finishes with O((N+M)*D):
  loss = (S@T + A) / (max(n_pairs,1) * max(n_rows,1))
    S: scatter-add of per-tile column counts into the global [M] array
    T[m] = W . yhat_m with W = sum_i [rc_i>0] * x_i/|x_i|  (from rc_out)
    A = sum(rc) - 2*sum(mc)
"""

import sys

sys.path.insert(0, "/opt/trn_rl_repo")

import numpy as np
from contextlib import ExitStack

N, M, D = 4096, 8192, 512
NCORES = 8
NL = N // NCORES          # 512 rows per core
NT = NL // 128            # 4 row tiles
KC = D // 128             # 4 contraction chunks
WT = 512                  # per-tile candidate window (measured max 395)
MARGIN = 3.0              # px window margin; safe vs fp32 d2 rounding

_cached = {}


def _split11(v):
    """Exact 2-piece split of fp32 into <=11-mantissa-bit halves."""
    v = np.asarray(v, np.float32)
    m, e = np.frexp(v)
    hi = np.ldexp(np.trunc(np.ldexp(m, 11)), e - 11).astype(np.float32)
    return hi, (v - hi).astype(np.float32)


def _split11_multi(v64, n):
    pieces = []
    rem = np.asarray(v64, np.float64)
    for _ in range(n):
        r32 = rem.astype(np.float32)
        m, e = np.frexp(r32)
        hi = np.ldexp(np.trunc(np.ldexp(m, 11)), e - 11).astype(np.float32)
        pieces.append(hi)
        rem = rem - hi.astype(np.float64)
    return pieces


def _mk_xpts(xp):
    x0h, x0l = _split11(xp[:, 0])
    x1h, x1l = _split11(xp[:, 1])
    one = np.ones(xp.shape[0], np.float32)
    # row k pairs with ypts row k: [y0h,y0l,y0h,y0l,y1h,y1l,y1h,y1l,yy1,yy2,yy3]
    return np.ascontiguousarray(np.stack(
        [-2 * x0h, -2 * x0h, -2 * x0l, -2 * x0l,
         -2 * x1h, -2 * x1h, -2 * x1l, -2 * x1l, one, one, one]))


def _mk_ypts(yp):
    y0h, y0l = _split11(yp[:, 0])
    y1h, y1l = _split11(yp[:, 1])
    yy64 = (yp[:, 0].astype(np.float64) ** 2 + yp[:, 1].astype(np.float64) ** 2)
    yy1, yy2, yy3 = _split11_multi(yy64, 3)
    return np.ascontiguousarray(
        np.stack([y0h, y0l, y0h, y0l, y1h, y1l, y1h, y1l, yy1, yy2, yy3]))


def _f8():
    import ml_dtypes
    return ml_dtypes.float8_e4m3fn


def _build_nc():
    from concourse import bacc, mybir, tile

    f32 = mybir.dt.float32
    f32r = mybir.dt.float32r
    bf16 = mybir.dt.bfloat16
    f8e4 = mybir.dt.float8e4
    nc = bacc.Bacc("TRN2", target_bir_lowering=False, debug=False)

    xdT8 = nc.dram_tensor("xdT8", [D, NL], f8e4, kind="ExternalInput")
    xpts = nc.dram_tensor("xpts", [11, NL], f32, kind="ExternalInput")
    thr = nc.dram_tensor("thr", [NL], f32, kind="ExternalInput")
    rx16 = nc.dram_tensor("rx16", [NL], f32, kind="ExternalInput")
    ypts4 = nc.dram_tensor("ypts4", [NT, 11, WT], f32, kind="ExternalInput")
    ydT84 = nc.dram_tensor("ydT84", [NT, D, WT], f8e4, kind="ExternalInput")

    S_out = nc.dram_tensor("S_out", [NT, WT], f32, kind="ExternalOutput")
    rc_out = nc.dram_tensor("rc_out", [NL], f32, kind="ExternalOutput")
    mc_out = nc.dram_tensor("mc_out", [NL], f32, kind="ExternalOutput")

    AF = mybir.ActivationFunctionType
    OP = mybir.AluOpType
    DR = mybir.MatmulPerfMode.DoubleRow

    with ExitStack() as ctx:
        tc = ctx.enter_context(tile.TileContext(nc))
        singles = ctx.enter_context(tc.tile_pool(name="singles", bufs=1))
        small = ctx.enter_context(tc.tile_pool(name="small", bufs=8))
        evac_pool = ctx.enter_context(tc.tile_pool(name="evac", bufs=2))
        ps_p = ctx.enter_context(tc.tile_pool(name="ps_p", bufs=2, space="PSUM"))
        ps_d = ctx.enter_context(tc.tile_pool(name="ps_d", bufs=2, space="PSUM"))
        ps_s = ctx.enter_context(tc.tile_pool(name="ps_s", bufs=1, space="PSUM"))

        # one-hot column weights: eye[t] is [128, NT] bf16 with column t = 1
        eyes = []
        for t in range(NT):
            e = singles.tile([128, NT], bf16, name=f"eye{t}", tag=f"eye{t}")
            nc.vector.memset(e, 0.0)
            nc.vector.memset(e[:, t:t + 1], 1.0)
            eyes.append(e)

        # ---- loads: mask-critical first ----
        sxpts = singles.tile([11, NL], f32r)
        nc.gpsimd.dma_start(out=sxpts, in_=xpts[:, :])
        syp = singles.tile([11, NT, WT], f32r)
        nc.gpsimd.dma_start(out=syp, in_=ypts4.rearrange("t k m -> k t m"))
        sthr = singles.tile([128, NT], f32)
        nc.sync.dma_start(out=sthr, in_=thr.rearrange("(t p) -> p t", p=128))
        sxdT = singles.tile([128, KC, NL], f8e4)
        nc.sync.dma_start(out=sxdT,
                          in_=xdT8[:, :].rearrange("(c p) n -> p c n", p=128))
        syd = []
        for t in range(NT):
            sydt = singles.tile([128, KC, WT], f8e4, name=f"syd{t}", tag=f"syd{t}")
            nc.sync.dma_start(
                out=sydt, in_=ydT84[t, :, :].rearrange("(c p) m -> p c m", p=128))
            syd.append(sydt)
        srx = singles.tile([128, NT], f32)
        nc.gpsimd.dma_start(out=srx, in_=rx16.rearrange("(t p) -> p t", p=128))

        rcst = singles.tile([128, NT], f32)
        mcst = singles.tile([128, NT], f32)
        mf_all = singles.tile([128, NT, WT], bf16)
        pS4 = ps_s.tile([NT, WT], f32)

        for t in range(NT):
            pp = ps_p.tile([128, WT], f32, name=f"pp{t}", tag="pp")
            nc.tensor.matmul(pp, sxpts[:, t * 128:(t + 1) * 128], syp[:, t, :],
                             start=True, stop=True)
            mf = mf_all[:, t, :]
            nc.vector.tensor_scalar(
                out=mf, in0=pp, scalar1=sthr[:, t:t + 1], scalar2=None,
                op0=OP.is_lt, op1=OP.add,
                accum_out=rcst[:, t:t + 1])

            pd = ps_d.tile([128, WT], f32, name=f"pd{t}", tag="pd")
            nc.tensor.matmul(pd, sxdT[:, 0:2, t * 128:(t + 1) * 128],
                             syd[t][:, 0:2, :], start=True, stop=False,
                             perf_mode=DR)
            nc.tensor.matmul(pd, sxdT[:, 2:4, t * 128:(t + 1) * 128],
                             syd[t][:, 2:4, :], start=False, stop=True,
                             perf_mode=DR)
            dummy = small.tile([128, 1], f32, name=f"dm{t}", tag="dm")
            nc.vector.affine_mul_reduce(
                out=dummy.broadcast_to(pd.shape),
                accum_out=mcst[:, t:t + 1],
                in0=pd, in1=mf, scale=srx[:, t:t + 1], bias=0.0)

        # column counts: accumulate all 4 tiles into one [NT, WT] PSUM tile
        for t in range(NT):
            nc.tensor.matmul(pS4, eyes[t], mf_all[:, t, :], start=(t == 0),
                             stop=(t == NT - 1))
        sS4 = evac_pool.tile([NT, WT], f32, name="sS4", tag="sS")
        nc.scalar.activation(sS4, pS4, AF.Copy)
        nc.sync.dma_start(out=S_out[:, :], in_=sS4)

        nc.sync.dma_start(out=rc_out.rearrange("(t p) -> p t", p=128), in_=rcst)
        nc.sync.dma_start(out=mc_out.rearrange("(t p) -> p t", p=128), in_=mcst)

    nc.finalize()
    return nc


def _get_nc():
    if "nc" not in _cached:
        _cached["nc"] = _build_nc()
    return _cached["nc"]


def _make_in_maps(valid_pts_scr, mem_pts_scr, valid_desc, mem_desc):
    """Returns (in_maps, meta); meta carries window offsets + host y/x data."""
    f8 = _f8()
    vp = np.asarray(valid_pts_scr, np.float32)
    mp = np.asarray(mem_pts_scr, np.float32)
    vd = np.asarray(valid_desc, np.float32)
    md = np.asarray(mem_desc, np.float32)

    xs = np.argsort(vp[:, 0], kind="stable")
    ms = np.argsort(mp[:, 0], kind="stable")
    vp_s, vd_s = vp[xs], vd[xs]
    mp_s, md_s = mp[ms], md[ms]

    yy = np.sum(md_s.astype(np.float64) ** 2, -1)
    ry = 1.0 / np.sqrt(yy)
    yhat = (md_s * ry[:, None]).astype(np.float32)                 # [M, D]
    ydT8_full = np.ascontiguousarray((yhat * 16.0).T.astype(f8))   # [D, M]
    ypts_full = _mk_ypts(mp_s)                                     # [11, M]

    xx_pt = (vp_s[:, 0].astype(np.float64) ** 2
             + vp_s[:, 1].astype(np.float64) ** 2)
    xx_d = np.sum(vd_s.astype(np.float64) ** 2, -1)
    rx = 1.0 / np.sqrt(xx_d)

    in_maps, offs = [], []
    for c in range(NCORES):
        rows = slice(c * NL, (c + 1) * NL)
        xp = vp_s[rows]
        ypts4 = np.empty((NT, 11, WT), np.float32)
        ydT84 = np.empty((NT, D, WT), f8)
        offs_c = []
        for t in range(NT):
            tp = xp[t * 128:(t + 1) * 128]
            lo = tp[:, 0].min() - MARGIN
            hi = tp[:, 0].max() + MARGIN
            s = int(np.searchsorted(mp_s[:, 0], lo, side="left"))
            e = int(np.searchsorted(mp_s[:, 0], hi, side="right"))
            assert e - s <= WT, f"core {c} tile {t}: window {e - s} > WT {WT}"
            s = min(s, M - WT)
            offs_c.append(s)
            ypts4[t] = ypts_full[:, s:s + WT]
            ydT84[t] = ydT8_full[:, s:s + WT]
        offs.append(offs_c)

        in_maps.append({
            "xdT8": np.ascontiguousarray(vd_s[rows].T.astype(f8)),
            "xpts": _mk_xpts(xp),
            "thr": (4.0 - xx_pt[rows]).astype(np.float32),
            "rx16": (rx[rows] / 16.0).astype(np.float32),
            "ypts4": ypts4,
            "ydT84": np.ascontiguousarray(ydT84),
        })
    meta = {"offs": offs, "yhat": yhat, "vd_s": vd_s, "rx": rx}
    return in_maps, meta


def _finish(results, meta):
    offs = meta["offs"]
    yhat = meta["yhat"]
    Stot = np.zeros(M, np.float64)
    A = 0.0
    nrows = 0.0
    rh_all = np.zeros(N, np.float64)
    for c in range(NCORES):
        r = results[c]
        S4 = r["S_out"].astype(np.float64)
        for t in range(NT):
            s = offs[c][t]
            Stot[s:s + WT] += S4[t]
        rc = r["rc_out"].astype(np.float64)
        mc = r["mc_out"].astype(np.float64)
        A += rc.sum() - 2.0 * mc.sum()
        nrows += float((rc > 0).sum())
        rh_all[c * NL:(c + 1) * NL] = rc > 0
    npairs = Stot.sum()
    if nrows > 0:
        W = ((rh_all * meta["rx"])[:, None] * meta["vd_s"]).sum(0)  # [D]
        T = yhat.astype(np.float64) @ W
        loss = (Stot @ T + A) / (max(npairs, 1.0) * max(nrows, 1.0))
    else:
        loss = 0.0
    return np.float32(loss)


def kernel(valid_pts_scr, mem_pts_scr, valid_desc, mem_desc):
    from concourse.bass_utils import run_bass_kernel_spmd

    in_maps, meta = _make_in_maps(valid_pts_scr, mem_pts_scr,
                                  valid_desc, mem_desc)
    nc = _get_nc()
    res = run_bass_kernel_spmd(nc, in_maps, core_ids=list(range(NCORES)))
    _cached["last_results"] = res
    return _finish(res.results, meta)
